# revision 34
# baseline (speedup 1.0000x reference)
"""Trainium2 Bass kernel for nn_NeighbourAggregation (gnn_message_passing).

Full-input contract: kernel(states[4096,8] f32, log_tau scalar f32) -> [4096,12] f32.

Strategy (8 cores, shard the query dim i into 8 slices of 512):
  Per query row i the reference reduces algebraically to:
    dist[i,j] = sqrt(|p_i - p_j|^2 + eps),  W = exp(-dist/tau + shift), W[i,i] = 0
    s1 = W @ [pos,vel] / rowsum(W),  s2 = W @ [pos^2,vel^2] / rowsum(W)
    mu = c_i - s1,  sigma = sqrt(s2 - s1^2 + 1e-6)      (i-offsets cancel)
    group_vel = mean(vel),  vel_dev = vel - group_vel
  Device schedule per core (tiles laid out [j=128 partitions, i=512 free]):
    - d2 via PE matmul, fp16 hi/lo split operands (K=10), with +3e-5 injected
      through the |p_i|^2 rank-1 term so d2 > 0 always (no NaN clamp needed)
    - dist = sqrt(d2) split across engines per 1024-column quarter:
      a few quarters on ACT (sqrt table), the rest on DVE via a bit-trick
      rsqrt seed (one int32-domain tensor_scalar on DVE or GpSimd computes
      bitcast(int32(float(bits(d2))*-0.5 + K)) ~ rsqrt(d2)) followed by one
      fused Newton step in a custom 5-stage DVE op:
      dist = d2*r0*(A - B*d2*r0^2), constants tuned to 7.5e-4 max rel err
    - W = exp(-dist/tau + ln(1000)) on ACT; the shift cancels in the softmax
      ratio and keeps W in fp16 normal range
    - diagonal W zeroed by a mask multiply on DVE; per-core j-chunks rotated
      so the diagonal lands in chunks 0..3 (same NEFF on all cores)
    - moments via PE matmul, W fp16 x [Dhi|Dlo] fp16, fp32 PSUM accumulation,
      split in two accumulators so most of the copy-out overlaps the tail
    - group_vel via two tiny PE matmuls over a host-packed vel^T/N tile,
      computed during startup
    - finalize in transposed layout: merge+transpose matmuls per 128-query
      chunk, per-partition-scalar DVE/GpSimd ops; sigma sqrt reuses the ACT
      sqrt table reloaded right after the last exp
"""

import sys

sys.path.insert(0, "/opt/trn_rl_repo")

import numpy as np

import concourse.bass as bass
import concourse.mybir as mybir
import concourse.tile as tile
from concourse import bacc
from concourse import bass_utils
from concourse.tile_rust import add_dep_helper
import concourse.dve_ops as dve_ops
from concourse.dve_ops import DveOp
from concourse.dve_spec import Spec, Src0, Src1, C0, C1

F32 = mybir.dt.float32
F16 = mybir.dt.float16
I32 = mybir.dt.int32
AF = mybir.ActivationFunctionType
ALU = mybir.AluOpType

N = 4096
NCORES = 8
NI = N // NCORES          # 512 queries per core
P = 128                   # partitions
NCHUNK = N // P           # 32 j-chunks
NQ = 16                   # 1024-column dist quarters (2 chunks each)
EXP_SHIFT = float(np.log(1000.0))  # logit shift, cancels in softmax
EPS_BIG = 3e-5            # injected into |p_i|^2 so PE-rounded d2 stays > 0

# fast-sqrt constants (seed K, Newton A/B); tuned against the exact f32
# rounding chain of the seed op, max rel err 7.5e-4 over all fp32 scales
SEED_K = 1597024128.0
NR_A = 1.549271835102724
NR_B = 0.5500832313302817

# per-quarter dist engine: 'a' = ACT sqrt, 'd' = DVE-seeded NR,
# 'p' = GpSimd-seeded NR
MODES = ["a" if q < 7 else "d" for q in range(16)]

# ---- custom DVE op: one Newton step for sqrt ------------------------------
_t = Src0 * Src1
_u = _t * Src1
_NR_BODY = _t * (C0 - _u * C1)


def _ref_sqrt_nr(in0, in1, c0, c1, c2):
    t = (in0 * in1).astype(np.float32)
    u = (t * in1).astype(np.float32)
    return (t * (np.float32(c0)
                 - (u * np.float32(c1)).astype(np.float32)).astype(np.float32)
            ).astype(np.float32)


SQRT_NR_ANT = DveOp(
    "SQRT_NR_ANT",
    Spec(body=_NR_BODY, reference=_ref_sqrt_nr),
    subdim=False,
    uops_sha={"v3": "6c90a41433774265"},
)

if SQRT_NR_ANT.name not in [o.name for o in dve_ops.OPS]:
    dve_ops.OPS.append(SQRT_NR_ANT)
    dve_ops._SUB_OPCODE_FOR_NAME[SQRT_NR_ANT.name] = (
        dve_ops._CUSTOM_DVE_ROW_BASE + len(dve_ops.OPS) - 1)
    dve_ops.CUSTOM_DVE_SPECS[SQRT_NR_ANT.name] = SQRT_NR_ANT.spec

_BUILT = None


def _build_bass():
    nc = bacc.Bacc(
        "TRN2",
        target_bir_lowering=False,
        debug=False,
        enable_asserts=False,
    )

    def din(name, shape, dt=F32):
        return nc.dram_tensor(name, shape, dt, kind="ExternalInput").ap()

    statj = din("statj", [10, N], F16)
    movi = din("movi", [10, NI], F16)
    dmom = din("dmom", [P, NCHUNK * 18], F16)
    diagmask = din("diagmask", [P, 4 * NI], F16)
    velmm = din("velmm", [P, 65], F16)   # vel^T/N chunked [.,0:64] + ones col
    selvxy = din("selvxy", [64, 2])
    cpack = din("cpack", [P, 24])        # ct4t [.,0:16] + ctvt [.,16:24]
    apack = din("apack", [P, 3])         # actscale, actbias, 1e-6
    selmerge = din("selmerge", [18, 9])  # [I9; I9]
    ones128 = din("ones128", [1, P])
    out_d = nc.dram_tensor("out", [NI, 12], F32, kind="ExternalOutput").ap()

    with tile.TileContext(nc) as tc:
        with (
            tc.tile_pool(name="consts", bufs=1) as consts,
            tc.tile_pool(name="dist", bufs=1) as distpool,
            tc.tile_pool(name="seeds", bufs=4) as seedpool,
            tc.tile_pool(name="w", bufs=2) as wpool,
            tc.tile_pool(name="fin", bufs=1) as fin,
        ):
            statj_sb = consts.tile([10, N], F16)
            movi_sb = consts.tile([10, NI], F16)
            apack_sb = consts.tile([P, 3], F32)
            dmom_sb = consts.tile([P, NCHUNK * 18], F16)
            diagmask_sb = consts.tile([P, 4 * NI], F16)
            cpack_sb = consts.tile([P, 24], F32)
            velmm_sb = consts.tile([P, 65], F16)
            selvxy_sb = consts.tile([64, 2], F32)
            selmerge_sb = consts.tile([18, 9], F32)
            ones128_sb = consts.tile([1, P], F32)

            # scratch memsets first: PE warm-up + sqrt-table trigger fire
            # in the first ~0.3us
            dummy = fin.tile([1, 1], F32, tag="dummy")
            nc.gpsimd.memset(dummy[:], 1.0)
            scr16 = fin.tile([1, 1], F16, tag="scr16")
            nc.gpsimd.memset(scr16[:], 1.0)
            nc.scalar.activation(dummy[:], dummy[:], AF.Sqrt, bias=0.0)

            # input DMAs on the SP queue, most-urgent first
            for sb, dr in [
                (statj_sb[0:10, 0:512], statj[0:10, 0:512]),
                (movi_sb, movi),
                (statj_sb[0:10, 512:N], statj[0:10, 512:N]),
                (velmm_sb, velmm),
                (selvxy_sb, selvxy), (ones128_sb, ones128),
                (diagmask_sb, diagmask), (apack_sb, apack), (dmom_sb, dmom),
                (cpack_sb, cpack), (selmerge_sb, selmerge),
            ]:
                nc.sync.dma_start(sb[:], dr[:])

            dist_all = distpool.tile([P, N * 4], F32)   # [128, 16384]

            psB = tc.tile_pool(name="psB", bufs=1, space="PSUM")
            psBp = psB.__enter__()
            psM2 = psBp.tile([50, NI], F32, tag="psM2")
            psMa = psM2[0:18, :]
            psMb = psM2[32:50, :]
            psG = tc.tile_pool(name="psG", bufs=1, space="PSUM")
            psGp = psG.__enter__()
            psA = tc.tile_pool(name="psA", bufs=3, space="PSUM")
            psAp = psA.__enter__()

            # ---- group_vel part 1: one PE matmul during startup --------
            # one PSUM bank holds all three little gv targets
            psgt = psGp.tile([P, 5], F32, tag="psgt")
            nc.tensor.matmul(psgt[0:64, 0:1], lhsT=velmm_sb[:, 0:64],
                             rhs=velmm_sb[:, 64:65], start=True, stop=True)

            # ---- dist production + interleaved exp/moments -------------
            sqrt_insts = []
            mm_t = 0
            NSPLIT = 24

            def moments(w, k):
                nonlocal mm_t
                tgt = psMa if mm_t < NSPLIT else psMb
                nc.tensor.matmul(
                    tgt,
                    lhsT=dmom_sb[:, mm_t * 18:(mm_t + 1) * 18],
                    rhs=w[:, (k % 8) * NI:((k % 8) + 1) * NI],
                    start=(mm_t in (0, NSPLIT)),
                    stop=(mm_t in (NSPLIT - 1, NCHUNK - 1)),
                )
                mm_t += 1

            w_tiles = [wpool.tile([P, N], F16, tag=f"w{g}", name=f"w{g}")
                       for g in range(4)]
            exp_insts = []

            def emit_exp(g, lo, hi):
                """exp over dist_all column range [lo, hi) into group g's W."""
                ei = nc.scalar.activation(
                    w_tiles[g][:, lo - g * N:hi - g * N],
                    dist_all[:, lo:hi],
                    AF.Exp, bias=apack_sb[:, 1:2], scale=apack_sb[:, 0:1],
                )
                if sqrt_insts:
                    add_dep_helper(ei.ins, sqrt_insts[-1].ins, sync=False,
                                   reason="exp after ACT sqrts (table batch)")
                exp_insts.append(ei)
                return ei

            # exp pieces: (group, first-quarter, n-quarters)
            pieces = ([(0, q, 1) for q in range(4)]
                      + [(1, 4 + 2 * h, 2) for h in range(2)]
                      + [(2, 8 + 2 * h, 2) for h in range(2)]
                      + [(3, q, 1) for q in range(12, 16)])

            def emit_piece(j):
                g, q0, nq = pieces[j]
                emit_exp(g, q0 * 1024, (q0 + nq) * 1024)
                if j == 0:
                    nc.gpsimd.tensor_tensor(
                        out=w_tiles[0][:, 0:1024], in0=w_tiles[0][:, 0:1024],
                        in1=diagmask_sb[:, 0:1024], op=ALU.mult)
                if j == 1:
                    nc.gpsimd.tensor_tensor(
                        out=w_tiles[0][:, 1024:2048],
                        in0=w_tiles[0][:, 1024:2048],
                        in1=diagmask_sb[:, 1024:2048], op=ALU.mult)
                for c in range(q0 * 2, (q0 + nq) * 2):
                    moments(w_tiles[g], c)
                if j == 7:
                    # early copy of the chunk-0..23 accumulator
                    nc.vector.tensor_copy(Mall_a[:], psMa)

            Mall_a = fin.tile([18, NI], F32)
            # ring order: DVE quarters first-alternating so the DVE dist
            # stream starts immediately; ACT sqrts spread between (they all
            # precede the exps in the ACT instruction stream regardless)
            ORDER = [0, 1, 2, 7, 3, 8, 4, 5, 9, 6, 10, 11, 12, 13, 14, 15]
            emitted_q = set()
            next_piece = 0

            def piece_ready(j):
                g, q0, nq = pieces[j]
                return (len(sqrt_insts) == MODES.count("a")
                        and all(qq in emitted_q for qq in range(q0, q0 + nq)))

            for slot, q in enumerate(ORDER):
                ps = psAp.tile([P, 1024], F32, tag="psA")
                if slot == 0:
                    for _ in range(6):   # PE p-state warm-up
                        nc.tensor.matmul(ps[0:1, 0:1], lhsT=scr16[:],
                                         rhs=scr16[:], start=True, stop=True)
                for c in range(2):
                    t = q * 2 + c
                    nc.tensor.matmul(
                        ps[:, c * NI:(c + 1) * NI],
                        lhsT=statj_sb[:, t * P:(t + 1) * P],
                        rhs=movi_sb[:],
                        start=True, stop=True,
                    )
                dpart = dist_all[:, q * 1024:(q + 1) * 1024]
                if MODES[q] == "a":
                    si = nc.scalar.activation(dpart, ps[:], AF.Sqrt, bias=0.0)
                    sqrt_insts.append(si)
                else:
                    seed = seedpool.tile([P, 1024], I32, tag="seed")
                    nc.vector.tensor_scalar(
                        out=seed[:], in0=ps[:].bitcast(I32),
                        scalar1=-0.5, scalar2=SEED_K,
                        op0=ALU.mult, op1=ALU.add)
                    nc.vector._custom_dve(
                        SQRT_NR_ANT, out=dpart, in0=ps[:],
                        in1=seed[:].bitcast(F32), s0=NR_A, s1=NR_B)
                emitted_q.add(q)
                # at most one exp/moment piece per slot keeps the PE stream
                # in execution order
                if next_piece < len(pieces) and piece_ready(next_piece):
                    emit_piece(next_piece)
                    next_piece += 1

            psA.__exit__(None, None, None)

            for j in range(next_piece, len(pieces)):
                emit_piece(j)

            # ---- group_vel part 2 (ACT copies in its idle tail) --------
            gpart = fin.tile([64, 1], F32, tag="gpart")
            nc.scalar.copy(gpart[:], psgt[0:64, 0:1])
            nc.tensor.matmul(psgt[0:1, 1:3], lhsT=gpart[:], rhs=selvxy_sb[:],
                             start=True, stop=True)
            growv = fin.tile([1, 2], F32, tag="growv")
            nc.scalar.copy(growv[:], psgt[0:1, 1:3])
            nc.tensor.matmul(psgt[:, 3:5], lhsT=ones128_sb[:], rhs=growv[:],
                             start=True, stop=True)
            gvb = fin.tile([P, 2], F32, tag="gvb")
            nc.scalar.copy(gvb[:], psgt[:, 3:5])

            # reload the sqrt table right after the last exp (overlaps the
            # moment tail; sigma then costs ~0.2us)
            dummy2 = fin.tile([1, 1], F32, tag="dummy2")
            nc.gpsimd.memset(dummy2[:], 1.0)
            s2i = nc.scalar.activation(dummy2[:], dummy2[:], AF.Sqrt, bias=0.0)
            add_dep_helper(s2i.ins, exp_insts[-1].ins, sync=False,
                           reason="sqrt table reload after last exp")

            # ---- finalize (transposed layout) --------------------------
            Mall_b = fin.tile([18, NI], F32)
            nc.vector.tensor_copy(Mall_b[:], psMb)
            psG.__exit__(None, None, None)
            psB.__exit__(None, None, None)

            psFpool = tc.tile_pool(name="psF", bufs=1, space="PSUM")
            psF = psFpool.__enter__()

            ot = fin.tile([P, 48], F32, tag="ot")
            sg_all = fin.tile([P, 16], F32, tag="sg")
            sge = fin.tile([P, 16], F32, tag="sge")
            sgseed = fin.tile([P, 16], I32, tag="sgseed")
            ot3 = ot[:].rearrange("p (k d) -> p k d", d=12)
            for k in range(4):
                psT = psF.tile([P, 9], F32, tag=f"psT{k}")
                nc.tensor.matmul(psT[:], lhsT=Mall_a[:, k * P:(k + 1) * P],
                                 rhs=selmerge_sb[:], start=True, stop=False)
                nc.tensor.matmul(psT[:], lhsT=Mall_b[:, k * P:(k + 1) * P],
                                 rhs=selmerge_sb[:], start=False, stop=True)
                rinv = fin.tile([P, 1], F32, tag=f"rinv{k}")
                nc.vector.reciprocal_approx_fast(rinv[:], psT[:, 8:9])
                s_k = fin.tile([P, 8], F32, tag=f"s{k}")
                nc.scalar.activation(s_k[:], psT[:, 0:8], AF.Copy,
                                     scale=rinv[:])
                # mu = c - s1  (Pool)
                nc.gpsimd.tensor_tensor(
                    out=ot3[:, k, 0:4], in0=cpack_sb[:, 4 * k:4 * k + 4],
                    in1=s_k[:, 0:4], op=ALU.subtract)
                # sig2 = s2 - s1^2  (DVE)
                t2 = fin.tile([P, 4], F32, tag=f"t2{k}")
                nc.vector.tensor_tensor(out=t2[:], in0=s_k[:, 0:4],
                                        in1=s_k[:, 0:4], op=ALU.mult)
                nc.vector.tensor_tensor(out=sg_all[:, 4 * k:4 * k + 4],
                                        in0=s_k[:, 4:8], in1=t2[:],
                                        op=ALU.subtract)
                # vel_dev + group_vel columns (Pool)
                nc.gpsimd.tensor_tensor(
                    out=ot3[:, k, 10:12],
                    in0=cpack_sb[:, 16 + 2 * k:16 + 2 * k + 2],
                    in1=gvb[:], op=ALU.subtract)
                nc.gpsimd.tensor_copy(ot3[:, k, 8:10], gvb[:])

            # sigma + store in two halves on parallel queues
            out_rr = out_d.rearrange("(k p) d -> p k d", p=P)
            nc.scalar.activation(
                ot3[:, 0:2, 4:8],
                sg_all[:, 0:8].rearrange("p (k d) -> p k d", d=4),
                AF.Sqrt, bias=apack_sb[:, 2:3])
            nc.scalar.dma_start(out_rr[:, 0:2, :], ot3[:, 0:2, :])
            nc.scalar.activation(
                ot3[:, 2:4, 4:8],
                sg_all[:, 8:16].rearrange("p (k d) -> p k d", d=4),
                AF.Sqrt, bias=apack_sb[:, 2:3])
            nc.sync.dma_start(out_rr[:, 2:4, :], ot3[:, 2:4, :])
            psFpool.__exit__(None, None, None)

    nc.finalize()
    return nc


def _host_prep(states, log_tau):
    states = np.asarray(states, dtype=np.float32)
    tau = np.exp(np.float32(log_tau)).astype(np.float32)
    pos = ((states[:, :2] + states[:, 2:4]) / 2.0).astype(np.float32)
    vel = ((states[:, 4:6] + states[:, 6:8]) / 2.0).astype(np.float32)
    p2 = (pos[:, 0] * pos[:, 0] + pos[:, 1] * pos[:, 1]).astype(np.float32)
    p2i = (p2 + np.float32(EPS_BIG)).astype(np.float32)

    f16 = np.float16
    ph = pos.astype(f16)
    pl = (pos - ph.astype(np.float32)).astype(f16)
    p2h = p2.astype(f16)
    p2l = (p2 - p2h.astype(np.float32)).astype(f16)
    p2ih = p2i.astype(f16)
    p2il = (p2i - p2ih.astype(np.float32)).astype(f16)

    C = np.concatenate([pos, vel], axis=1).astype(np.float32)          # [N,4]
    D = np.concatenate([C, C * C, np.ones((N, 1), np.float32)], 1)     # [N,9]
    Dh = D.astype(f16)
    Dl = (D - Dh.astype(np.float32)).astype(f16)

    ones_n = np.ones(N, f16)
    diagmask = np.ones((P, 4 * NI), f16)
    pp = np.arange(P)
    for k in range(4):
        diagmask[pp, k * NI + P * k + pp] = 0.0

    selmerge = np.concatenate([np.eye(9)] * 2, 0).astype(np.float32)
    velsc = (vel / np.float32(N)).astype(np.float32)
    velmm = np.zeros((P, 65), f16)
    velmm[:, 0:64] = velsc.reshape(NCHUNK, P, 2).transpose(1, 0, 2).reshape(P, 64)
    velmm[:, 64] = 1.0
    selvxy = np.zeros((64, 2), np.float32)
    selvxy[0::2, 0] = 1.0
    selvxy[1::2, 1] = 1.0

    in_maps = []
    for c in range(NCORES):
        # j-chunk rotation: device chunk t holds original chunk (t + 4c) % 32
        jperm = np.concatenate(
            [np.arange(((t + 4 * c) % NCHUNK) * P, ((t + 4 * c) % NCHUNK) * P + P)
             for t in range(NCHUNK)]
        )
        isl = np.arange(NI * c, NI * (c + 1))

        statj_a = np.stack([
            ph[jperm, 0], ph[jperm, 1], pl[jperm, 0], pl[jperm, 1],
            ph[jperm, 0], ph[jperm, 1], p2h[jperm], p2l[jperm],
            ones_n[:N], ones_n[:N],
        ]).astype(f16)                                                 # [10, N]
        m2 = np.float16(-2.0)
        movi_a = np.stack([
            m2 * ph[isl, 0], m2 * ph[isl, 1], m2 * ph[isl, 0], m2 * ph[isl, 1],
            m2 * pl[isl, 0], m2 * pl[isl, 1], ones_n[:NI], ones_n[:NI],
            p2ih[isl], p2il[isl],
        ]).astype(f16)                                                 # [10, NI]

        dmom_a = np.empty((P, NCHUNK * 18), f16)
        Dhp = Dh[jperm].reshape(NCHUNK, P, 9)
        Dlp = Dl[jperm].reshape(NCHUNK, P, 9)
        for t in range(NCHUNK):
            dmom_a[:, t * 18:t * 18 + 9] = Dhp[t]
            dmom_a[:, t * 18 + 9:t * 18 + 18] = Dlp[t]

        ct4t = C[isl].reshape(4, P, 4).transpose(1, 0, 2).reshape(P, 16)
        ctvt = vel[isl].reshape(4, P, 2).transpose(1, 0, 2).reshape(P, 8)
        cpack = np.concatenate([ct4t, ctvt], axis=1).astype(np.float32)

        apack = np.stack([
            np.full(P, -1.0 / tau, np.float32),
            np.full(P, EXP_SHIFT, np.float32),
            np.full(P, 1e-6, np.float32),
        ], axis=1)

        in_maps.append({
            "statj": statj_a,
            "movi": movi_a,
            "dmom": dmom_a,
            "diagmask": diagmask,
            "velmm": velmm,
            "selvxy": selvxy,
            "cpack": cpack,
            "apack": apack,
            "selmerge": selmerge,
            "ones128": np.ones((1, P), np.float32),
        })
    return in_maps


def _get_built():
    global _BUILT
    if _BUILT is None:
        _BUILT = _build_bass()
    return _BUILT


def kernel(states, log_tau, _trace=False, _trace_kwargs=None):
    nc = _get_built()
    in_maps = _host_prep(states, log_tau)
    res = bass_utils.run_bass_kernel_spmd(
        nc, in_maps, core_ids=list(range(NCORES)),
        trace=_trace, **(_trace_kwargs or {}),
    )
    out = np.concatenate([res.results[c]["out"] for c in range(NCORES)], axis=0)
    if _trace:
        kernel._last_results = res
    return out.astype(np.float32)


# revision 35
# speedup vs baseline: 1.0102x; 1.0102x over previous
"""Trainium2 Bass kernel for nn_NeighbourAggregation (gnn_message_passing).

Full-input contract: kernel(states[4096,8] f32, log_tau scalar f32) -> [4096,12] f32.

Strategy (8 cores, shard the query dim i into 8 slices of 512):
  Per query row i the reference reduces algebraically to:
    dist[i,j] = sqrt(|p_i - p_j|^2 + eps),  W = exp(-dist/tau + shift), W[i,i] = 0
    s1 = W @ [pos,vel] / rowsum(W),  s2 = W @ [pos^2,vel^2] / rowsum(W)
    mu = c_i - s1,  sigma = sqrt(s2 - s1^2 + 1e-6)      (i-offsets cancel)
    group_vel = mean(vel),  vel_dev = vel - group_vel
  Device schedule per core (tiles laid out [j=128 partitions, i=512 free]):
    - d2 via PE matmul, fp16 hi/lo split operands (K=10), with +3e-5 injected
      through the |p_i|^2 rank-1 term so d2 > 0 always (no NaN clamp needed)
    - dist = sqrt(d2) split across engines per 1024-column quarter:
      a few quarters on ACT (sqrt table), the rest on DVE via a bit-trick
      rsqrt seed (one int32-domain tensor_scalar on DVE or GpSimd computes
      bitcast(int32(float(bits(d2))*-0.5 + K)) ~ rsqrt(d2)) followed by one
      fused Newton step in a custom 5-stage DVE op:
      dist = d2*r0*(A - B*d2*r0^2), constants tuned to 7.5e-4 max rel err
    - W = exp(-dist/tau + ln(1000)) on ACT; the shift cancels in the softmax
      ratio and keeps W in fp16 normal range
    - diagonal W zeroed by a mask multiply on DVE; per-core j-chunks rotated
      so the diagonal lands in chunks 0..3 (same NEFF on all cores)
    - moments via PE matmul, W fp16 x [Dhi|Dlo] fp16, fp32 PSUM accumulation,
      split in two accumulators so most of the copy-out overlaps the tail
    - group_vel via two tiny PE matmuls over a host-packed vel^T/N tile,
      computed during startup
    - finalize in transposed layout: merge+transpose matmuls per 128-query
      chunk, per-partition-scalar DVE/GpSimd ops; sigma sqrt reuses the ACT
      sqrt table reloaded right after the last exp
"""

import sys

sys.path.insert(0, "/opt/trn_rl_repo")

import numpy as np

import concourse.bass as bass
import concourse.mybir as mybir
import concourse.tile as tile
from concourse import bacc
from concourse import bass_utils
from concourse.tile_rust import add_dep_helper
import concourse.dve_ops as dve_ops
from concourse.dve_ops import DveOp
from concourse.dve_spec import Spec, Src0, Src1, C0, C1

F32 = mybir.dt.float32
F16 = mybir.dt.float16
I32 = mybir.dt.int32
AF = mybir.ActivationFunctionType
ALU = mybir.AluOpType

N = 4096
NCORES = 8
NI = N // NCORES          # 512 queries per core
P = 128                   # partitions
NCHUNK = N // P           # 32 j-chunks
NQ = 16                   # 1024-column dist quarters (2 chunks each)
EXP_SHIFT = float(np.log(1000.0))  # logit shift, cancels in softmax
EPS_BIG = 3e-5            # injected into |p_i|^2 so PE-rounded d2 stays > 0

# fast-sqrt constants (seed K, Newton A/B); tuned against the exact f32
# rounding chain of the seed op, max rel err 7.5e-4 over all fp32 scales
SEED_K = 1597024128.0
NR_A = 1.549271835102724
NR_B = 0.5500832313302817

# per-quarter dist engine: 'a' = ACT sqrt, 'd' = DVE-seeded NR,
# 'p' = GpSimd-seeded NR
MODES = ["a" if q < 7 else "d" for q in range(16)]

# ---- custom DVE op: one Newton step for sqrt ------------------------------
_t = Src0 * Src1
_u = _t * Src1
_NR_BODY = _t * (C0 - _u * C1)


def _ref_sqrt_nr(in0, in1, c0, c1, c2):
    t = (in0 * in1).astype(np.float32)
    u = (t * in1).astype(np.float32)
    return (t * (np.float32(c0)
                 - (u * np.float32(c1)).astype(np.float32)).astype(np.float32)
            ).astype(np.float32)


SQRT_NR_ANT = DveOp(
    "SQRT_NR_ANT",
    Spec(body=_NR_BODY, reference=_ref_sqrt_nr),
    subdim=False,
    uops_sha={"v3": "6c90a41433774265"},
)

if SQRT_NR_ANT.name not in [o.name for o in dve_ops.OPS]:
    dve_ops.OPS.append(SQRT_NR_ANT)
    dve_ops._SUB_OPCODE_FOR_NAME[SQRT_NR_ANT.name] = (
        dve_ops._CUSTOM_DVE_ROW_BASE + len(dve_ops.OPS) - 1)
    dve_ops.CUSTOM_DVE_SPECS[SQRT_NR_ANT.name] = SQRT_NR_ANT.spec

_BUILT = None


def _build_bass():
    nc = bacc.Bacc(
        "TRN2",
        target_bir_lowering=False,
        debug=False,
        enable_asserts=False,
    )

    def din(name, shape, dt=F32):
        return nc.dram_tensor(name, shape, dt, kind="ExternalInput").ap()

    statj = din("statj", [10, N], F16)
    movi = din("movi", [10, NI], F16)
    dmom = din("dmom", [P, NCHUNK * 18], F16)
    diagmask = din("diagmask", [P, 4 * NI], F16)
    velmm = din("velmm", [P, 65], F16)   # vel^T/N chunked [.,0:64] + ones col
    selvxy = din("selvxy", [64, 2])
    cpack = din("cpack", [P, 24])        # ct4t [.,0:16] + ctvt [.,16:24]
    apack = din("apack", [P, 3])         # actscale, actbias, 1e-6
    selmerge = din("selmerge", [18, 9])  # [I9; I9]
    ones128 = din("ones128", [1, P])
    out_d = nc.dram_tensor("out", [NI, 12], F32, kind="ExternalOutput").ap()

    with tile.TileContext(nc) as tc:
        with (
            tc.tile_pool(name="consts", bufs=1) as consts,
            tc.tile_pool(name="dist", bufs=1) as distpool,
            tc.tile_pool(name="seeds", bufs=4) as seedpool,
            tc.tile_pool(name="w", bufs=2) as wpool,
            tc.tile_pool(name="fin", bufs=1) as fin,
        ):
            statj_sb = consts.tile([10, N], F16)
            movi_sb = consts.tile([10, NI], F16)
            apack_sb = consts.tile([P, 3], F32)
            dmom_sb = consts.tile([P, NCHUNK * 18], F16)
            diagmask_sb = consts.tile([P, 4 * NI], F16)
            cpack_sb = consts.tile([P, 24], F32)
            velmm_sb = consts.tile([P, 65], F16)
            selvxy_sb = consts.tile([64, 2], F32)
            selmerge_sb = consts.tile([18, 9], F32)
            ones128_sb = consts.tile([1, P], F32)

            # scratch memsets first: PE warm-up + sqrt-table trigger fire
            # in the first ~0.3us
            dummy = fin.tile([1, 1], F32, tag="dummy")
            nc.gpsimd.memset(dummy[:], 1.0)
            scr16 = fin.tile([1, 1], F16, tag="scr16")
            nc.gpsimd.memset(scr16[:], 1.0)
            nc.scalar.activation(dummy[:], dummy[:], AF.Sqrt, bias=0.0)

            # input DMAs on the SP queue, most-urgent first
            for sb, dr in [
                (statj_sb[0:10, 0:512], statj[0:10, 0:512]),
                (movi_sb, movi),
                (statj_sb[0:10, 512:N], statj[0:10, 512:N]),
                (velmm_sb, velmm),
                (selvxy_sb, selvxy), (ones128_sb, ones128),
                (diagmask_sb, diagmask), (apack_sb, apack), (dmom_sb, dmom),
                (cpack_sb, cpack), (selmerge_sb, selmerge),
            ]:
                nc.sync.dma_start(sb[:], dr[:])

            dist_all = distpool.tile([P, N * 4], F32)   # [128, 16384]

            psB = tc.tile_pool(name="psB", bufs=1, space="PSUM")
            psBp = psB.__enter__()
            psM2 = psBp.tile([50, NI], F32, tag="psM2")
            psMa = psM2[0:18, :]
            psMb = psM2[32:50, :]
            psG = tc.tile_pool(name="psG", bufs=1, space="PSUM")
            psGp = psG.__enter__()
            psA = tc.tile_pool(name="psA", bufs=3, space="PSUM")
            psAp = psA.__enter__()

            # ---- group_vel part 1: one PE matmul during startup --------
            # one PSUM bank holds all three little gv targets
            psgt = psGp.tile([P, 5], F32, tag="psgt")
            nc.tensor.matmul(psgt[0:64, 0:1], lhsT=velmm_sb[:, 0:64],
                             rhs=velmm_sb[:, 64:65], start=True, stop=True)

            # ---- dist production + interleaved exp/moments -------------
            sqrt_insts = []
            mm_t = 0
            NSPLIT = 24

            def moments(w, k):
                nonlocal mm_t
                tgt = psMa if mm_t < NSPLIT else psMb
                nc.tensor.matmul(
                    tgt,
                    lhsT=dmom_sb[:, mm_t * 18:(mm_t + 1) * 18],
                    rhs=w[:, (k % 8) * NI:((k % 8) + 1) * NI],
                    start=(mm_t in (0, NSPLIT)),
                    stop=(mm_t in (NSPLIT - 1, NCHUNK - 1)),
                )
                mm_t += 1

            w_tiles = [wpool.tile([P, N], F16, tag=f"w{g}", name=f"w{g}")
                       for g in range(4)]
            exp_insts = []

            def emit_exp(g, lo, hi):
                """exp over dist_all column range [lo, hi) into group g's W."""
                ei = nc.scalar.activation(
                    w_tiles[g][:, lo - g * N:hi - g * N],
                    dist_all[:, lo:hi],
                    AF.Exp, bias=apack_sb[:, 1:2], scale=apack_sb[:, 0:1],
                )
                if sqrt_insts:
                    add_dep_helper(ei.ins, sqrt_insts[-1].ins, sync=False,
                                   reason="exp after ACT sqrts (table batch)")
                exp_insts.append(ei)
                return ei

            # exp pieces: (group, first-quarter, n-quarters)
            pieces = ([(0, q, 1) for q in range(4)]
                      + [(1, 4 + 2 * h, 2) for h in range(2)]
                      + [(2, 8 + 2 * h, 2) for h in range(2)]
                      + [(3, q, 1) for q in range(12, 16)])

            def emit_piece(j):
                g, q0, nq = pieces[j]
                emit_exp(g, q0 * 1024, (q0 + nq) * 1024)
                if j == 0:
                    nc.gpsimd.tensor_tensor(
                        out=w_tiles[0][:, 0:1024], in0=w_tiles[0][:, 0:1024],
                        in1=diagmask_sb[:, 0:1024], op=ALU.mult)
                if j == 1:
                    nc.gpsimd.tensor_tensor(
                        out=w_tiles[0][:, 1024:2048],
                        in0=w_tiles[0][:, 1024:2048],
                        in1=diagmask_sb[:, 1024:2048], op=ALU.mult)
                for c in range(q0 * 2, (q0 + nq) * 2):
                    moments(w_tiles[g], c)
                if j == 7:
                    # early copy of the chunk-0..23 accumulator
                    nc.vector.tensor_copy(Mall_a[:], psMa)

            Mall_a = fin.tile([18, NI], F32)
            # ring order: DVE quarters first-alternating so the DVE dist
            # stream starts immediately; ACT sqrts spread between (they all
            # precede the exps in the ACT instruction stream regardless)
            ORDER = [0, 1, 2, 7, 3, 8, 4, 5, 9, 6, 10, 11, 12, 13, 14, 15]
            emitted_q = set()
            next_piece = 0

            def piece_ready(j):
                g, q0, nq = pieces[j]
                return (len(sqrt_insts) == MODES.count("a")
                        and all(qq in emitted_q for qq in range(q0, q0 + nq)))

            for slot, q in enumerate(ORDER):
                ps = psAp.tile([P, 1024], F32, tag="psA")
                if slot == 0:
                    for _ in range(6):   # PE p-state warm-up
                        nc.tensor.matmul(ps[0:1, 0:1], lhsT=scr16[:],
                                         rhs=scr16[:], start=True, stop=True)
                for c in range(2):
                    t = q * 2 + c
                    nc.tensor.matmul(
                        ps[:, c * NI:(c + 1) * NI],
                        lhsT=statj_sb[:, t * P:(t + 1) * P],
                        rhs=movi_sb[:],
                        start=True, stop=True,
                    )
                dpart = dist_all[:, q * 1024:(q + 1) * 1024]
                if MODES[q] == "a":
                    si = nc.scalar.activation(dpart, ps[:], AF.Sqrt, bias=0.0)
                    sqrt_insts.append(si)
                else:
                    seed = seedpool.tile([P, 1024], I32, tag="seed")
                    nc.vector.tensor_scalar(
                        out=seed[:], in0=ps[:].bitcast(I32),
                        scalar1=-0.5, scalar2=SEED_K,
                        op0=ALU.mult, op1=ALU.add)
                    nc.vector._custom_dve(
                        SQRT_NR_ANT, out=dpart, in0=ps[:],
                        in1=seed[:].bitcast(F32), s0=NR_A, s1=NR_B)
                emitted_q.add(q)
                # at most one exp/moment piece per slot keeps the PE stream
                # in execution order
                if next_piece < len(pieces) and piece_ready(next_piece):
                    emit_piece(next_piece)
                    next_piece += 1

            psA.__exit__(None, None, None)

            for j in range(next_piece, len(pieces)):
                emit_piece(j)

            # ---- group_vel part 2 (ACT copies in its idle tail) --------
            gpart = fin.tile([64, 1], F32, tag="gpart")
            nc.scalar.copy(gpart[:], psgt[0:64, 0:1])
            nc.tensor.matmul(psgt[0:1, 1:3], lhsT=gpart[:], rhs=selvxy_sb[:],
                             start=True, stop=True)
            growv = fin.tile([1, 2], F32, tag="growv")
            nc.scalar.copy(growv[:], psgt[0:1, 1:3])
            nc.tensor.matmul(psgt[:, 3:5], lhsT=ones128_sb[:], rhs=growv[:],
                             start=True, stop=True)
            gvb = fin.tile([P, 2], F32, tag="gvb")
            nc.scalar.copy(gvb[:], psgt[:, 3:5])

            # reload the sqrt table right after the last exp (overlaps the
            # moment tail; sigma then costs ~0.2us)
            dummy2 = fin.tile([1, 1], F32, tag="dummy2")
            nc.gpsimd.memset(dummy2[:], 1.0)
            s2i = nc.scalar.activation(dummy2[:], dummy2[:], AF.Sqrt, bias=0.0)
            add_dep_helper(s2i.ins, exp_insts[-1].ins, sync=False,
                           reason="sqrt table reload after last exp")

            # ---- finalize (transposed layout) --------------------------
            Mall_b = fin.tile([18, NI], F32)
            nc.vector.tensor_copy(Mall_b[:], psMb)
            psG.__exit__(None, None, None)
            psB.__exit__(None, None, None)

            psFpool = tc.tile_pool(name="psF", bufs=1, space="PSUM")
            psF = psFpool.__enter__()

            ot = fin.tile([P, 48], F32, tag="ot")
            sg_all = fin.tile([P, 16], F32, tag="sg")
            sge = fin.tile([P, 16], F32, tag="sge")
            sgseed = fin.tile([P, 16], I32, tag="sgseed")
            ot3 = ot[:].rearrange("p (k d) -> p k d", d=12)
            for k in range(4):
                psT = psF.tile([P, 9], F32, tag=f"psT{k}")
                nc.tensor.matmul(psT[:], lhsT=Mall_a[:, k * P:(k + 1) * P],
                                 rhs=selmerge_sb[:], start=True, stop=False)
                nc.tensor.matmul(psT[:], lhsT=Mall_b[:, k * P:(k + 1) * P],
                                 rhs=selmerge_sb[:], start=False, stop=True)
                rinv = fin.tile([P, 1], F32, tag=f"rinv{k}")
                nc.vector.reciprocal_approx_fast(rinv[:], psT[:, 8:9])
                s_k = fin.tile([P, 8], F32, tag=f"s{k}")
                nc.scalar.activation(s_k[:], psT[:, 0:8], AF.Copy,
                                     scale=rinv[:])
                # mu = c - s1  (Pool)
                nc.gpsimd.tensor_tensor(
                    out=ot3[:, k, 0:4], in0=cpack_sb[:, 4 * k:4 * k + 4],
                    in1=s_k[:, 0:4], op=ALU.subtract)
                # sig2 = s2 - s1^2  (DVE)
                t2 = fin.tile([P, 4], F32, tag=f"t2{k}")
                nc.vector.tensor_tensor(out=t2[:], in0=s_k[:, 0:4],
                                        in1=s_k[:, 0:4], op=ALU.mult)
                nc.vector.tensor_tensor(out=sg_all[:, 4 * k:4 * k + 4],
                                        in0=s_k[:, 4:8], in1=t2[:],
                                        op=ALU.subtract)
                # vel_dev + group_vel columns (Pool)
                nc.gpsimd.tensor_tensor(
                    out=ot3[:, k, 10:12],
                    in0=cpack_sb[:, 16 + 2 * k:16 + 2 * k + 2],
                    in1=gvb[:], op=ALU.subtract)
                nc.gpsimd.tensor_copy(ot3[:, k, 8:10], gvb[:])

            # sigma in two halves, then store on two parallel queues
            out_rr = out_d.rearrange("(k p) d -> p k d", p=P)
            nc.scalar.activation(
                ot3[:, 0:2, 4:8],
                sg_all[:, 0:8].rearrange("p (k d) -> p k d", d=4),
                AF.Sqrt, bias=apack_sb[:, 2:3])
            nc.sync.dma_start(out_rr[:, 0:2, :], ot3[:, 0:2, :])
            nc.scalar.activation(
                ot3[:, 2:4, 4:8],
                sg_all[:, 8:16].rearrange("p (k d) -> p k d", d=4),
                AF.Sqrt, bias=apack_sb[:, 2:3])
            nc.scalar.dma_start(out_rr[:, 2:4, :], ot3[:, 2:4, :])
            psFpool.__exit__(None, None, None)

    nc.finalize()
    return nc


def _host_prep(states, log_tau):
    states = np.asarray(states, dtype=np.float32)
    tau = np.exp(np.float32(log_tau)).astype(np.float32)
    pos = ((states[:, :2] + states[:, 2:4]) / 2.0).astype(np.float32)
    vel = ((states[:, 4:6] + states[:, 6:8]) / 2.0).astype(np.float32)
    p2 = (pos[:, 0] * pos[:, 0] + pos[:, 1] * pos[:, 1]).astype(np.float32)
    p2i = (p2 + np.float32(EPS_BIG)).astype(np.float32)

    f16 = np.float16
    ph = pos.astype(f16)
    pl = (pos - ph.astype(np.float32)).astype(f16)
    p2h = p2.astype(f16)
    p2l = (p2 - p2h.astype(np.float32)).astype(f16)
    p2ih = p2i.astype(f16)
    p2il = (p2i - p2ih.astype(np.float32)).astype(f16)

    C = np.concatenate([pos, vel], axis=1).astype(np.float32)          # [N,4]
    D = np.concatenate([C, C * C, np.ones((N, 1), np.float32)], 1)     # [N,9]
    Dh = D.astype(f16)
    Dl = (D - Dh.astype(np.float32)).astype(f16)

    ones_n = np.ones(N, f16)
    diagmask = np.ones((P, 4 * NI), f16)
    pp = np.arange(P)
    for k in range(4):
        diagmask[pp, k * NI + P * k + pp] = 0.0

    selmerge = np.concatenate([np.eye(9)] * 2, 0).astype(np.float32)
    velsc = (vel / np.float32(N)).astype(np.float32)
    velmm = np.zeros((P, 65), f16)
    velmm[:, 0:64] = velsc.reshape(NCHUNK, P, 2).transpose(1, 0, 2).reshape(P, 64)
    velmm[:, 64] = 1.0
    selvxy = np.zeros((64, 2), np.float32)
    selvxy[0::2, 0] = 1.0
    selvxy[1::2, 1] = 1.0

    in_maps = []
    for c in range(NCORES):
        # j-chunk rotation: device chunk t holds original chunk (t + 4c) % 32
        jperm = np.concatenate(
            [np.arange(((t + 4 * c) % NCHUNK) * P, ((t + 4 * c) % NCHUNK) * P + P)
             for t in range(NCHUNK)]
        )
        isl = np.arange(NI * c, NI * (c + 1))

        statj_a = np.stack([
            ph[jperm, 0], ph[jperm, 1], pl[jperm, 0], pl[jperm, 1],
            ph[jperm, 0], ph[jperm, 1], p2h[jperm], p2l[jperm],
            ones_n[:N], ones_n[:N],
        ]).astype(f16)                                                 # [10, N]
        m2 = np.float16(-2.0)
        movi_a = np.stack([
            m2 * ph[isl, 0], m2 * ph[isl, 1], m2 * ph[isl, 0], m2 * ph[isl, 1],
            m2 * pl[isl, 0], m2 * pl[isl, 1], ones_n[:NI], ones_n[:NI],
            p2ih[isl], p2il[isl],
        ]).astype(f16)                                                 # [10, NI]

        dmom_a = np.empty((P, NCHUNK * 18), f16)
        Dhp = Dh[jperm].reshape(NCHUNK, P, 9)
        Dlp = Dl[jperm].reshape(NCHUNK, P, 9)
        for t in range(NCHUNK):
            dmom_a[:, t * 18:t * 18 + 9] = Dhp[t]
            dmom_a[:, t * 18 + 9:t * 18 + 18] = Dlp[t]

        ct4t = C[isl].reshape(4, P, 4).transpose(1, 0, 2).reshape(P, 16)
        ctvt = vel[isl].reshape(4, P, 2).transpose(1, 0, 2).reshape(P, 8)
        cpack = np.concatenate([ct4t, ctvt], axis=1).astype(np.float32)

        apack = np.stack([
            np.full(P, -1.0 / tau, np.float32),
            np.full(P, EXP_SHIFT, np.float32),
            np.full(P, 1e-6, np.float32),
        ], axis=1)

        in_maps.append({
            "statj": statj_a,
            "movi": movi_a,
            "dmom": dmom_a,
            "diagmask": diagmask,
            "velmm": velmm,
            "selvxy": selvxy,
            "cpack": cpack,
            "apack": apack,
            "selmerge": selmerge,
            "ones128": np.ones((1, P), np.float32),
        })
    return in_maps


def _get_built():
    global _BUILT
    if _BUILT is None:
        _BUILT = _build_bass()
    return _BUILT


def kernel(states, log_tau, _trace=False, _trace_kwargs=None):
    nc = _get_built()
    in_maps = _host_prep(states, log_tau)
    res = bass_utils.run_bass_kernel_spmd(
        nc, in_maps, core_ids=list(range(NCORES)),
        trace=_trace, **(_trace_kwargs or {}),
    )
    out = np.concatenate([res.results[c]["out"] for c in range(NCORES)], axis=0)
    if _trace:
        kernel._last_results = res
    return out.astype(np.float32)
